# revision 24
# baseline (speedup 1.0000x reference)
"""Trainium2 Bass kernel for nn_ChunkedKasminaLayer (moe_routing).

Problem (B=8192, D_IN=1024, D_OUT=4096, S=64 seeds, C=64 chunk, H=128):
    base = x @ Wb.T + bb                     [B, D_OUT]
    y    = base.view(B, S, C)
    h    = relu(y @ W1[s] + b1[s])           per seed  [B, S, H]
    bp   = h @ W2[s] + b2[s]                 per seed  [B, S, C]
    out  = (1-a)*y + a*bp,  a = alpha*active (per seed)

Strategy: data-parallel over batch across 8 NeuronCores (1024 rows each,
weights replicated, no collectives). Everything is computed in transposed
layout (features on partitions, batch on the free axis, 512-wide tiles).

Key wins over the dense fp32r version (289us -> 210us HW):
  - Inactive seeds (a=0, pure passthrough) are SKIPPED: seeds are permuted
    host-side so active seeds pack into the first ceil(A/2) 128-row groups;
    those run the blueprint path, the rest go straight from the base PSUM
    through a bias-add to the output. The out DMA scatters each 64-row seed
    chunk back to its original row block.
  - All matmul operands are bf16 (walrus rejects mixed 32/16-bit inputs):
    InstMatmult is self-loading and the (unmodeled-by-the-cost-model)
    weight-load time scales with dtype width, so 2-byte operands halve the
    per-matmul load overhead vs fp32r. The passthrough blend term is
    computed from the f32 PSUM directly, so its precision is unaffected;
    rel err ~4e-3 vs the 2e-2 budget.
  - Passthrough groups are interleaved between blend groups: engine queues
    are in-order, so their independent base matmuls fill the PE stalls
    while a blend group waits on DVE/ACT intermediates.
  - x is loaded as per-k-tile chunks and the output is written bf16, so
    the first matmul waits on 256KB (not 2MB) and out DMA traffic halves.

Blend path per 128-row group (pair of active seeds):
  - base:  8 accumulating bf16 matmuls per [128, 512] tile (lhsT = Wb.T
           k-tile, rhs = x.T k-tile).
  - blend term (1-a)*(base+bb) + a*b2 on ACT straight from PSUM (f32).
  - L1:    pair run as two K=64 matmuls via tile_position (0,0)/(64,0),
           lhsT = W1[s]; relu+b1 fused into the PSUM->SBUF copy on the
           scalar engine (bf16 out); rhs yT = base+bb via DVE (bf16).
  - L2:    two bf16 matmuls with lhsT = alpha*W2[s] into the two
           64-partition halves of one PSUM group; final add on DVE.
"""

import numpy as np
import ml_dtypes

from concourse import bacc, mybir
from concourse import bass_utils
import concourse.tile as tile

B, D_IN, D_OUT, S = 8192, 1024, 4096, 64
C = D_OUT // S            # 64
H = 2 * C                 # 128
NCORES = 8
BC = B // NCORES          # 1024 batch rows per core
KT = D_IN // 128          # 8 k-tiles
MP = D_OUT // 128         # 32 m-tiles (= seed pairs after permutation)
NT = 512                  # batch tile (free dim)
NN = BC // NT             # 2 batch tiles per core

F32 = mybir.dt.float32
F32R = mybir.dt.float32r
BF16 = mybir.dt.bfloat16


def seed_order(active):
    """Permutation packing active seeds first; NBP = #blend groups."""
    act = np.asarray(active).astype(bool)
    order = np.argsort(~act, kind="stable")
    nbp = (int(act.sum()) + 1) // 2
    return order, nbp


def build(active, reps: int = 1, hw_loop: bool = False):
    """Build + compile the per-core Tile program (same program on all cores).

    The program is specialized on the active-seed mask: only the first NBP
    permuted groups run the blueprint MLP. reps>1 repeats the computation
    via a tc.For_i hardware loop for wall-clock slope timing.
    """
    order, NBP = seed_order(active)

    nc = bacc.Bacc("TRN2", target_bir_lowering=False, debug=False)

    t_xT = nc.dram_tensor("xT", [128, KT, BC], BF16, kind="ExternalInput")
    t_wbT = nc.dram_tensor("wbT", [128, KT, D_OUT], BF16, kind="ExternalInput")
    t_bb = nc.dram_tensor("bbT", [128, MP], F32, kind="ExternalInput")
    t_out = nc.dram_tensor("outT", [D_OUT, BC], BF16, kind="ExternalOutput")
    if NBP:
        t_w1 = nc.dram_tensor("w1s", [128, NBP, H], BF16, kind="ExternalInput")
        t_w2 = nc.dram_tensor("w2s", [H, NBP, 128], BF16, kind="ExternalInput")
        t_oma = nc.dram_tensor("omaT", [128, NBP], F32, kind="ExternalInput")
        t_b1 = nc.dram_tensor("b1T", [H, 2 * NBP], F32, kind="ExternalInput")
        # blb = oma*bb + alpha*b2: bias for the blend term computed from PSUM
        t_blb = nc.dram_tensor("blbT", [128, NBP], F32, kind="ExternalInput")

    with tile.TileContext(nc) as tc:
        with (
            tc.tile_pool(name="wres", bufs=1) as wres,
            tc.tile_pool(name="wbp", bufs=3) as wbp,
            tc.tile_pool(name="yp", bufs=3) as yp,
            tc.tile_pool(name="hp", bufs=4) as hp,
            tc.tile_pool(name="op", bufs=3) as op,
            tc.tile_pool(name="psy", bufs=2, space="PSUM") as psy,
            tc.tile_pool(name="psh", bufs=4, space="PSUM") as psh,
            tc.tile_pool(name="pso", bufs=2, space="PSUM") as pso,
        ):
            def body():
                # x split per k-tile so the first base matmul only waits for
                # its own 256KB chunk, not the whole 2MB transfer
                xk = [wres.tile([128, BC], BF16, tag=f"xs{k}", name=f"xk{k}")
                      for k in range(KT)]
                bb = wres.tile([128, MP], F32, tag="bb")
                nc.sync.dma_start(xk[0][:], t_xT.ap()[:, 0, :])
                loads = [(bb, t_bb)]
                if NBP:
                    w1 = wres.tile([128, NBP, H], BF16, tag="w1")
                    w2 = wres.tile([H, NBP, 128], BF16, tag="w2")
                    oma = wres.tile([128, NBP], F32, tag="oma")
                    b1 = wres.tile([H, 2 * NBP], F32, tag="b1")
                    blb = wres.tile([128, NBP], F32, tag="blb")
                    loads += [(w1, t_w1), (w2, t_w2), (oma, t_oma),
                              (b1, t_b1), (blb, t_blb)]
                for t, d in loads:
                    nc.sync.dma_start(t[:], d.ap())
                for k in range(1, KT):
                    nc.sync.dma_start(xk[k][:], t_xT.ap()[:, k, :])

                def emit_out(out_t, m, nsl):
                    # scatter the two 64-row seed chunks to original rows
                    w = (nsl.stop - nsl.start) if nsl else BC
                    c0 = nsl.start if nsl else 0
                    for j in range(2):
                        r0 = 64 * int(order[2 * m + j])
                        nc.sync.dma_start(
                            t_out.ap()[r0:r0 + 64, c0:c0 + w],
                            out_t[64 * j:64 * (j + 1), :])

                # Interleave passthrough groups between blend groups: engine
                # queues are in-order, so a blend group's L1/L2 matmuls stall
                # the PE on the DVE/ACT results; a passthrough group's base
                # matmuls are independent work that fills those stalls.
                seq, bi, fi = [], 0, NBP
                while bi < NBP or fi < MP:
                    if bi < NBP:
                        seq.append(bi); bi += 1
                    if fi < MP:
                        seq.append(fi); fi += 1

                for m in seq:
                    wb_m = wbp.tile([128, KT, 128], BF16)
                    nc.sync.dma_start(
                        wb_m[:], t_wbT.ap()[:, :, 128 * m:128 * (m + 1)])
                    for n in range(NN):
                        nsl = slice(NT * n, NT * (n + 1))
                        ps_y = psy.tile([128, NT], F32)
                        for k in range(KT):
                            nc.tensor.matmul(
                                ps_y[:], wb_m[:, k, :], xk[k][:, nsl],
                                start=(k == 0), stop=(k == KT - 1))

                        if m >= NBP:
                            out_t = op.tile([128, NT], BF16, tag="of")
                            nc.vector.tensor_scalar_add(
                                out_t[:], ps_y[:], bb[:, m:m + 1])
                            emit_out(out_t, m, nsl)
                            continue

                        bl = op.tile([128, NT], F32, tag="bl")
                        nc.scalar.activation(
                            bl[:], ps_y[:],
                            mybir.ActivationFunctionType.Identity,
                            bias=blb[:, m:m + 1], scale=oma[:, m:m + 1])

                        yT = yp.tile([128, NT], BF16)
                        nc.vector.tensor_scalar_add(yT[:], ps_y[:], bb[:, m:m + 1])

                        hs = []
                        for j in range(2):
                            psl = slice(64 * j, 64 * (j + 1))
                            ps_h = psh.tile([128, NT], F32)
                            nc.tensor.matmul(
                                ps_h[:], w1[psl, m, :], yT[psl, :],
                                start=True, stop=True, tile_position=(64 * j, 0))
                            h_j = hp.tile([128, NT], BF16)
                            nc.scalar.activation(
                                h_j[:], ps_h[:],
                                mybir.ActivationFunctionType.Relu,
                                bias=b1[:, 2 * m + j:2 * m + j + 1])
                            hs.append(h_j)

                        ps_o = pso.tile([128, NT], F32)
                        nc.tensor.matmul(ps_o[0:64, :], w2[:, m, 0:64], hs[0][:],
                                         start=True, stop=True)
                        nc.tensor.matmul(ps_o[64:128, :], w2[:, m, 64:128],
                                         hs[1][:], start=True, stop=True)
                        out_t = op.tile([128, NT], BF16, tag="ob")
                        nc.vector.tensor_tensor(out=out_t[:], in0=bl[:],
                                                in1=ps_o[:],
                                                op=mybir.AluOpType.add)
                        emit_out(out_t, m, nsl)

            if hw_loop and reps > 1:
                with tc.For_i(0, reps, 1):
                    body()
            else:
                for _ in range(reps):
                    body()

    nc.compile()
    return nc


def prep_shared(Wb, bb, W1, b1, W2, b2, alpha, active):
    """Host-side packing of the replicated (per-core-identical) inputs,
    permuted so active seeds come first."""
    order, NBP = seed_order(active)
    ae_f = (np.asarray(alpha).astype(np.float32)
            * np.asarray(active).astype(np.float32))

    Wb_p = np.asarray(Wb).reshape(S, C, D_IN)[order].reshape(D_OUT, D_IN)
    bb_p = np.asarray(bb).reshape(S, C)[order].reshape(D_OUT)
    ae = ae_f[order]
    W1_p = np.asarray(W1)[order]
    b1_p = np.asarray(b1)[order]
    W2_p = np.asarray(W2)[order]
    b2_p = np.asarray(b2)[order]

    wbT = np.ascontiguousarray(
        Wb_p.T.reshape(KT, 128, D_OUT).transpose(1, 0, 2)
    ).astype(ml_dtypes.bfloat16)
    bbT = np.ascontiguousarray(bb_p.reshape(MP, 128).T).astype(np.float32)
    out = {"wbT": wbT, "bbT": bbT}
    if NBP == 0:
        return out

    SB = 2 * NBP  # seeds covered by blend groups
    w1s = np.ascontiguousarray(
        W1_p[:SB].reshape(NBP, 128, H).transpose(1, 0, 2)
    ).astype(ml_dtypes.bfloat16)

    W2p = (ae[:SB, None, None] * W2_p[:SB]).astype(np.float32)   # [SB, H, C]
    w2s = np.ascontiguousarray(
        W2p.reshape(NBP, 2, H, C).transpose(2, 0, 1, 3).reshape(H, NBP, 128)
    ).astype(ml_dtypes.bfloat16)

    omaT = np.ascontiguousarray(
        np.repeat(1.0 - ae[:SB], C).astype(np.float32).reshape(NBP, 128).T)
    b1T = np.ascontiguousarray(b1_p[:SB].T).astype(np.float32)   # [H, SB]
    b2p = (ae[:SB, None] * b2_p[:SB]).astype(np.float32)         # [SB, C]
    b2T = np.ascontiguousarray(b2p.reshape(NBP, 128).T)
    blbT = (omaT * bbT[:, :NBP] + b2T).astype(np.float32)

    out.update({"w1s": w1s, "w2s": w2s, "omaT": omaT, "b1T": b1T, "blbT": blbT})
    return out


def prep_core(x_shard):
    """x_shard [BC, D_IN] -> xT [128, KT, BC] bf16."""
    return np.ascontiguousarray(
        x_shard.T.reshape(KT, 128, BC).transpose(1, 0, 2)
    ).astype(ml_dtypes.bfloat16)


def run(nc, in_maps):
    import time
    last = None
    for attempt in range(4):
        try:
            return bass_utils.run_bass_kernel_spmd(
                nc, in_maps, core_ids=list(range(NCORES)))
        except Exception as e:  # transient NRT_EXEC_UNIT_UNRECOVERABLE etc.
            last = e
            time.sleep(20 * (attempt + 1))
    raise last


def kernel(x, Wb, bb, W1, b1, W2, b2, alpha, active):
    nc = build(active, 1)
    shared = prep_shared(Wb, bb, W1, b1, W2, b2, alpha, active)
    in_maps = [
        {**shared, "xT": prep_core(x[i * BC:(i + 1) * BC])}
        for i in range(NCORES)
    ]
    res = run(nc, in_maps)
    out = np.empty((B, D_OUT), np.float32)
    for i in range(NCORES):
        out[i * BC:(i + 1) * BC] = res.results[i]["outT"].astype(np.float32).T
    return out


# revision 25
# speedup vs baseline: 1.0013x; 1.0013x over previous
"""Trainium2 Bass kernel for nn_ChunkedKasminaLayer (moe_routing).

Problem (B=8192, D_IN=1024, D_OUT=4096, S=64 seeds, C=64 chunk, H=128):
    base = x @ Wb.T + bb                     [B, D_OUT]
    y    = base.view(B, S, C)
    h    = relu(y @ W1[s] + b1[s])           per seed  [B, S, H]
    bp   = h @ W2[s] + b2[s]                 per seed  [B, S, C]
    out  = (1-a)*y + a*bp,  a = alpha*active (per seed)

Strategy: data-parallel over batch across 8 NeuronCores (1024 rows each,
weights replicated, no collectives). Everything is computed in transposed
layout (features on partitions, batch on the free axis, 512-wide tiles).

Key wins over the dense fp32r version (289us -> 210us HW):
  - Inactive seeds (a=0, pure passthrough) are SKIPPED: seeds are permuted
    host-side so active seeds pack into the first ceil(A/2) 128-row groups;
    those run the blueprint path, the rest go straight from the base PSUM
    through a bias-add to the output. The out DMA scatters each 64-row seed
    chunk back to its original row block.
  - All matmul operands are bf16 (walrus rejects mixed 32/16-bit inputs):
    InstMatmult is self-loading and the (unmodeled-by-the-cost-model)
    weight-load time scales with dtype width, so 2-byte operands halve the
    per-matmul load overhead vs fp32r. The passthrough blend term is
    computed from the f32 PSUM directly, so its precision is unaffected;
    rel err ~4e-3 vs the 2e-2 budget.
  - Passthrough groups are interleaved between blend groups: engine queues
    are in-order, so their independent base matmuls fill the PE stalls
    while a blend group waits on DVE/ACT intermediates.
  - x is loaded as per-k-tile chunks and the output is written bf16, so
    the first matmul waits on 256KB (not 2MB) and out DMA traffic halves.

Blend path per 128-row group (pair of active seeds):
  - base:  8 accumulating bf16 matmuls per [128, 512] tile (lhsT = Wb.T
           k-tile, rhs = x.T k-tile).
  - blend term (1-a)*(base+bb) + a*b2 on ACT straight from PSUM (f32).
  - L1:    pair run as two K=64 matmuls via tile_position (0,0)/(64,0),
           lhsT = W1[s]; relu+b1 fused into the PSUM->SBUF copy on the
           scalar engine (bf16 out); rhs yT = base+bb via DVE (bf16).
  - L2:    two bf16 matmuls with lhsT = alpha*W2[s] into the two
           64-partition halves of one PSUM group; final add on DVE.
"""

import numpy as np
import ml_dtypes

from concourse import bacc, mybir
from concourse import bass_utils
import concourse.tile as tile

B, D_IN, D_OUT, S = 8192, 1024, 4096, 64
C = D_OUT // S            # 64
H = 2 * C                 # 128
NCORES = 8
BC = B // NCORES          # 1024 batch rows per core
KT = D_IN // 128          # 8 k-tiles
MP = D_OUT // 128         # 32 m-tiles (= seed pairs after permutation)
NT = 512                  # batch tile (free dim)
NN = BC // NT             # 2 batch tiles per core

F32 = mybir.dt.float32
F32R = mybir.dt.float32r
BF16 = mybir.dt.bfloat16


def seed_order(active):
    """Permutation packing active seeds first; NBP = #blend groups."""
    act = np.asarray(active).astype(bool)
    order = np.argsort(~act, kind="stable")
    nbp = (int(act.sum()) + 1) // 2
    return order, nbp


def build(active, reps: int = 1, hw_loop: bool = False):
    """Build + compile the per-core Tile program (same program on all cores).

    The program is specialized on the active-seed mask: only the first NBP
    permuted groups run the blueprint MLP. reps>1 repeats the computation
    via a tc.For_i hardware loop for wall-clock slope timing.
    """
    order, NBP = seed_order(active)

    nc = bacc.Bacc("TRN2", target_bir_lowering=False, debug=False)

    t_xT = nc.dram_tensor("xT", [128, KT, BC], BF16, kind="ExternalInput")
    t_wbT = nc.dram_tensor("wbT", [128, KT, D_OUT], BF16, kind="ExternalInput")
    t_bb = nc.dram_tensor("bbT", [128, MP], F32, kind="ExternalInput")
    t_out = nc.dram_tensor("outT", [D_OUT, BC], BF16, kind="ExternalOutput")
    if NBP:
        t_w1 = nc.dram_tensor("w1s", [128, NBP, H], BF16, kind="ExternalInput")
        t_w2 = nc.dram_tensor("w2s", [H, NBP, 128], BF16, kind="ExternalInput")
        t_oma = nc.dram_tensor("omaT", [128, NBP], F32, kind="ExternalInput")
        t_b1 = nc.dram_tensor("b1T", [H, 2 * NBP], F32, kind="ExternalInput")
        # blb = oma*bb + alpha*b2: bias for the blend term computed from PSUM
        t_blb = nc.dram_tensor("blbT", [128, NBP], F32, kind="ExternalInput")

    with tile.TileContext(nc) as tc:
        with (
            tc.tile_pool(name="wres", bufs=1) as wres,
            tc.tile_pool(name="wbp", bufs=3) as wbp,
            tc.tile_pool(name="yp", bufs=3) as yp,
            tc.tile_pool(name="hp", bufs=4) as hp,
            tc.tile_pool(name="op", bufs=3) as op,
            tc.tile_pool(name="psy", bufs=3, space="PSUM") as psy,
            tc.tile_pool(name="psh", bufs=3, space="PSUM") as psh,
            tc.tile_pool(name="pso", bufs=2, space="PSUM") as pso,
        ):
            def body():
                # x split per k-tile so the first base matmul only waits for
                # its own 256KB chunk, not the whole 2MB transfer
                xk = [wres.tile([128, BC], BF16, tag=f"xs{k}", name=f"xk{k}")
                      for k in range(KT)]
                bb = wres.tile([128, MP], F32, tag="bb")
                nc.sync.dma_start(xk[0][:], t_xT.ap()[:, 0, :])
                loads = [(bb, t_bb)]
                if NBP:
                    w1 = wres.tile([128, NBP, H], BF16, tag="w1")
                    w2 = wres.tile([H, NBP, 128], BF16, tag="w2")
                    oma = wres.tile([128, NBP], F32, tag="oma")
                    b1 = wres.tile([H, 2 * NBP], F32, tag="b1")
                    blb = wres.tile([128, NBP], F32, tag="blb")
                    loads += [(w1, t_w1), (w2, t_w2), (oma, t_oma),
                              (b1, t_b1), (blb, t_blb)]
                for t, d in loads:
                    nc.sync.dma_start(t[:], d.ap())
                for k in range(1, KT):
                    nc.sync.dma_start(xk[k][:], t_xT.ap()[:, k, :])

                def emit_out(out_t, m, nsl):
                    # scatter the two 64-row seed chunks to original rows
                    w = (nsl.stop - nsl.start) if nsl else BC
                    c0 = nsl.start if nsl else 0
                    for j in range(2):
                        r0 = 64 * int(order[2 * m + j])
                        nc.sync.dma_start(
                            t_out.ap()[r0:r0 + 64, c0:c0 + w],
                            out_t[64 * j:64 * (j + 1), :])

                # Interleave passthrough groups between blend groups: engine
                # queues are in-order, so a blend group's L1/L2 matmuls stall
                # the PE on the DVE/ACT results; a passthrough group's base
                # matmuls are independent work that fills those stalls.
                seq, bi, fi = [], 0, NBP
                while bi < NBP or fi < MP:
                    if bi < NBP:
                        seq.append(bi); bi += 1
                    if fi < MP:
                        seq.append(fi); fi += 1

                for m in seq:
                    wb_m = wbp.tile([128, KT, 128], BF16)
                    nc.sync.dma_start(
                        wb_m[:], t_wbT.ap()[:, :, 128 * m:128 * (m + 1)])
                    for n in range(NN):
                        nsl = slice(NT * n, NT * (n + 1))
                        ps_y = psy.tile([128, NT], F32)
                        for k in range(KT):
                            nc.tensor.matmul(
                                ps_y[:], wb_m[:, k, :], xk[k][:, nsl],
                                start=(k == 0), stop=(k == KT - 1))

                        if m >= NBP:
                            out_t = op.tile([128, NT], BF16, tag="of")
                            nc.vector.tensor_scalar_add(
                                out_t[:], ps_y[:], bb[:, m:m + 1])
                            emit_out(out_t, m, nsl)
                            continue

                        bl = op.tile([128, NT], F32, tag="bl")
                        nc.scalar.activation(
                            bl[:], ps_y[:],
                            mybir.ActivationFunctionType.Identity,
                            bias=blb[:, m:m + 1], scale=oma[:, m:m + 1])

                        yT = yp.tile([128, NT], BF16)
                        nc.vector.tensor_scalar_add(yT[:], ps_y[:], bb[:, m:m + 1])

                        hs = []
                        for j in range(2):
                            psl = slice(64 * j, 64 * (j + 1))
                            ps_h = psh.tile([128, NT], F32)
                            nc.tensor.matmul(
                                ps_h[:], w1[psl, m, :], yT[psl, :],
                                start=True, stop=True, tile_position=(64 * j, 0))
                            h_j = hp.tile([128, NT], BF16)
                            nc.scalar.activation(
                                h_j[:], ps_h[:],
                                mybir.ActivationFunctionType.Relu,
                                bias=b1[:, 2 * m + j:2 * m + j + 1])
                            hs.append(h_j)

                        ps_o = pso.tile([128, NT], F32)
                        nc.tensor.matmul(ps_o[0:64, :], w2[:, m, 0:64], hs[0][:],
                                         start=True, stop=True)
                        nc.tensor.matmul(ps_o[64:128, :], w2[:, m, 64:128],
                                         hs[1][:], start=True, stop=True)
                        out_t = op.tile([128, NT], BF16, tag="ob")
                        nc.vector.tensor_tensor(out=out_t[:], in0=bl[:],
                                                in1=ps_o[:],
                                                op=mybir.AluOpType.add)
                        emit_out(out_t, m, nsl)

            if hw_loop and reps > 1:
                with tc.For_i(0, reps, 1):
                    body()
            else:
                for _ in range(reps):
                    body()

    nc.compile()
    return nc


def prep_shared(Wb, bb, W1, b1, W2, b2, alpha, active):
    """Host-side packing of the replicated (per-core-identical) inputs,
    permuted so active seeds come first."""
    order, NBP = seed_order(active)
    ae_f = (np.asarray(alpha).astype(np.float32)
            * np.asarray(active).astype(np.float32))

    Wb_p = np.asarray(Wb).reshape(S, C, D_IN)[order].reshape(D_OUT, D_IN)
    bb_p = np.asarray(bb).reshape(S, C)[order].reshape(D_OUT)
    ae = ae_f[order]
    W1_p = np.asarray(W1)[order]
    b1_p = np.asarray(b1)[order]
    W2_p = np.asarray(W2)[order]
    b2_p = np.asarray(b2)[order]

    wbT = np.ascontiguousarray(
        Wb_p.T.reshape(KT, 128, D_OUT).transpose(1, 0, 2)
    ).astype(ml_dtypes.bfloat16)
    bbT = np.ascontiguousarray(bb_p.reshape(MP, 128).T).astype(np.float32)
    out = {"wbT": wbT, "bbT": bbT}
    if NBP == 0:
        return out

    SB = 2 * NBP  # seeds covered by blend groups
    w1s = np.ascontiguousarray(
        W1_p[:SB].reshape(NBP, 128, H).transpose(1, 0, 2)
    ).astype(ml_dtypes.bfloat16)

    W2p = (ae[:SB, None, None] * W2_p[:SB]).astype(np.float32)   # [SB, H, C]
    w2s = np.ascontiguousarray(
        W2p.reshape(NBP, 2, H, C).transpose(2, 0, 1, 3).reshape(H, NBP, 128)
    ).astype(ml_dtypes.bfloat16)

    omaT = np.ascontiguousarray(
        np.repeat(1.0 - ae[:SB], C).astype(np.float32).reshape(NBP, 128).T)
    b1T = np.ascontiguousarray(b1_p[:SB].T).astype(np.float32)   # [H, SB]
    b2p = (ae[:SB, None] * b2_p[:SB]).astype(np.float32)         # [SB, C]
    b2T = np.ascontiguousarray(b2p.reshape(NBP, 128).T)
    blbT = (omaT * bbT[:, :NBP] + b2T).astype(np.float32)

    out.update({"w1s": w1s, "w2s": w2s, "omaT": omaT, "b1T": b1T, "blbT": blbT})
    return out


def prep_core(x_shard):
    """x_shard [BC, D_IN] -> xT [128, KT, BC] bf16."""
    return np.ascontiguousarray(
        x_shard.T.reshape(KT, 128, BC).transpose(1, 0, 2)
    ).astype(ml_dtypes.bfloat16)


def run(nc, in_maps):
    import time
    last = None
    for attempt in range(4):
        try:
            return bass_utils.run_bass_kernel_spmd(
                nc, in_maps, core_ids=list(range(NCORES)))
        except Exception as e:  # transient NRT_EXEC_UNIT_UNRECOVERABLE etc.
            last = e
            time.sleep(20 * (attempt + 1))
    raise last


def kernel(x, Wb, bb, W1, b1, W2, b2, alpha, active):
    nc = build(active, 1)
    shared = prep_shared(Wb, bb, W1, b1, W2, b2, alpha, active)
    in_maps = [
        {**shared, "xT": prep_core(x[i * BC:(i + 1) * BC])}
        for i in range(NCORES)
    ]
    res = run(nc, in_maps)
    out = np.empty((B, D_OUT), np.float32)
    for i in range(NCORES):
        out[i * BC:(i + 1) * BC] = res.results[i]["outT"].astype(np.float32).T
    return out
